# revision 14
# baseline (speedup 1.0000x reference)
"""Multi-head attention (B=2, S=2048, D=2048, H=16, RoPE, causal) on 8 TRN2 cores.

Sharding: tensor-parallel over heads (2 heads/core) x batch as data.  Each core:
  phase 1: qkv projection for its 2 heads (both batches), RoPE fused into drain.
           qT,kT produced transposed [Dh, S]; v produced natural [S, Dh].
  phase 2: causal attention per (b,h) pair: s^T = kT.T @ qT blocks -> exp ->
           mask -> oT += v.T @ pT, row-sums l += ones.T @ quad-sums (PSUM).
  phase 3: partial out-proj: out_partial = sum_h diag(1/l_h) oT_h.T @ Wout_h.
Host sums the 8 partial outputs and adds b_out.

v6 design notes (evolved from the 353.6us v3 baseline):
  - v3's phase 2+3 was SCALAR-bound (exp + drain copies ~148us of a 155us
    window).  v6: causal-restricted diagonal blocks (-20% attn PE, -15%
    exp), exp per PAIR of key blocks over [128,1024] 2-bank psum tiles,
    triangle mask via gpsimd affine_select, strip zeroing via gpsimd
    memset, quad-sum tree (pair adds on DVE, quad adds on gpsimd) with one
    ones-matmul per quad, and out-proj drains 512-wide split ACT 3/10 /
    DVE 7/10.
  - out-proj is a work QUEUE of [128,512] psum fills (ps3 bufs=2 so fills
    pipeline behind drains) pumped 1-2 items between attention pairs --
    an earlier serial version let proj back up behind attention and ran
    the tail 50us at HAM half clock.
  - the early DMA feed is round-robin shared across in-flight transfers
    and weight tensors moved at 2KB/partition descriptor lines (~40-80
    GB/s).  v6 packs wq|wk|wv quarter-major into ONE dram tensor (6KB
    lines, one transfer per quarter) and the rope/bias tables into two
    packed tensors; tcn0 runs quarter-major (4 k-steps per chain per
    quarter) so real matmuls start at ~12us instead of ~21us.
"""

import numpy as np
import ml_dtypes

B, S, D = 2, 2048, 2048
H, DH = 16, 128
NCORES = 8
HPC = H // NCORES          # heads per core
T = B * S                  # 4096 tokens
SCALE = 1.0 / float(np.sqrt(DH))
ROPE_BASE = 10000.0

TC_N = T // 512            # 8 token chunks of 512 (phase 1)
KT_N = D // 128            # 16 contraction tiles
JB_N = S // 128            # 16 key blocks per sequence
IC_N = S // 512            # 4 query chunks per sequence
TT_N = S // 128            # 16 token tiles per batch (phase 3)
NC_N = D // 512            # 4 out-column chunks

_CACHE = {}


def _build_program():
    import concourse.bacc as bacc
    import concourse.mybir as mybir
    import concourse.tile as tile
    import concourse.bass as bass

    f32 = mybir.dt.float32
    bf16 = mybir.dt.bfloat16
    add = mybir.AluOpType.add
    mult = mybir.AluOpType.mult
    is_ge = mybir.AluOpType.is_ge
    Exp = mybir.ActivationFunctionType.Exp
    Copy = mybir.ActivationFunctionType.Copy
    Ident = mybir.ActivationFunctionType.Identity
    PSUM = bass.MemorySpace.PSUM

    nc = bacc.Bacc("TRN2", target_bir_lowering=False, debug=False)

    # partition-major x: row tcn*128+p holds token chunk tcn's per-partition
    # line [k, 512] (16KB contiguous per partition -> fat DMA descriptors)
    xT = nc.dram_tensor("xT", [TC_N * 128, KT_N * 512], bf16, kind="ExternalInput")
    # all projection weights, quarter-major: quarter kq occupies cols
    # [kq*3072,(kq+1)*3072) = [wq 1024 | wk 1024 | wv 1024], so one DMA per
    # quarter moves 6KB/partition contiguous lines
    wqkv = nc.dram_tensor("wqkv", [128, 4 * 3072], bf16, kind="ExternalInput")
    wo = nc.dram_tensor("wo", [HPC * DH, D], bf16, kind="ExternalInput")
    # packed tables: bf16 [cos 2048 | sin 2048]; f32 [bq 2 | bk 2 | bvb 256]
    tbf = nc.dram_tensor("tbf", [128, 4096], bf16, kind="ExternalInput")
    tf32 = nc.dram_tensor("tf32", [128, 260], f32, kind="ExternalInput")
    out = nc.dram_tensor("out", [T, D], bf16, kind="ExternalOutput")

    def wq_sl(k, h):
        kq, j = divmod(k, 4)
        c = kq * 3072 + j * 256 + h * 128
        return slice(c, c + 128)

    def wk_sl(k, h):
        kq, j = divmod(k, 4)
        c = kq * 3072 + 1024 + j * 256 + h * 128
        return slice(c, c + 128)

    def wv_sl(k):
        kq, j = divmod(k, 4)
        c = kq * 3072 + 2048 + j * 256
        return slice(c, c + 256)

    with tile.TileContext(nc) as tc:
        with tc.tile_pool(name="persist", bufs=1) as pp:
            # --- resident weights/constants ---
            wqkv_sb = pp.tile([128, 4 * 3072], bf16, tag="wqkv_sb", name="wqkv_sb")
            wo_sb = pp.tile([128, HPC * D], bf16, tag="wo_sb", name="wo_sb")
            tbf_sb = pp.tile([128, 4096], bf16, tag="tbf_sb", name="tbf_sb")
            tf32_sb = pp.tile([128, 260], f32, tag="tf32_sb", name="tf32_sb")
            cos_sb = tbf_sb[:, 0:2048]
            sin_sb = tbf_sb[:, 2048:4096]
            bq_sb = tf32_sb[:, 0:HPC]
            bk_sb = tf32_sb[:, HPC:2 * HPC]
            bvb_sb = tf32_sb[:, 2 * HPC:2 * HPC + HPC * DH]
            # all-ones stationary: ones128.T @ t replicates colsums to all
            # 128 PSUM partitions -> denominator tile needs no broadcast
            ones_sb = pp.tile([128, 128], bf16, tag="ones_sb", name="ones_sb")
            nc.vector.memset(ones_sb[:], 1.0)

            # --- per-(b,h) persistent tensors ---
            qT, kT, vN, oT = {}, {}, {}, {}
            for b in range(B):
                for h in range(HPC):
                    qT[b, h] = pp.tile([128, S], bf16, tag=f"qT{b}{h}", name=f"qT{b}{h}")
                    kT[b, h] = pp.tile([128, S], bf16, tag=f"kT{b}{h}", name=f"kT{b}{h}")
                    vN[b, h] = pp.tile([128, S], bf16, tag=f"vN{b}{h}", name=f"vN{b}{h}")
                    oT[b, h] = pp.tile([128, S], bf16, tag=f"oT{b}{h}", name=f"oT{b}{h}")

            # ================= phase 1: qkv projection =================
            with tc.tile_pool(name="xtp", bufs=4) as xtp, \
                 tc.tile_pool(name="ps_qk", bufs=4, space=PSUM) as ps_qk, \
                 tc.tile_pool(name="ps_v", bufs=4, space=PSUM) as ps_v, \
                 tc.tile_pool(name="rtp", bufs=4) as rtp:
                # tcn0 arrives as 4 quarter-chunks; chains run quarter-major
                # so the PE starts as soon as quarter 0 lands.
                xt0 = xtp.tile([128, KT_N * 512], bf16, tag="xt", name="xt0")
                for kq in range(4):
                    nc.sync.dma_start(
                        xt0[:, kq * 2048:(kq + 1) * 2048],
                        xT[0:128, kq * 2048:(kq + 1) * 2048])
                # HAM warm-up bridging the NEFF preamble (~8us) to the first
                # quarter of data (~12us); real chains then take over.
                pwm = ps_v.tile([128, 512], f32, tag="psv", name="pwm")
                for _ in range(64):
                    nc.tensor.matmul(pwm[:, 0:128], ones_sb[:], ones_sb[:],
                                     start=True, stop=True)
                # scalar ring: one fat transfer per weight quarter
                for kq in range(4):
                    nc.scalar.dma_start(
                        wqkv_sb[:, kq * 3072:(kq + 1) * 3072],
                        wqkv[:, kq * 3072:(kq + 1) * 3072])
                # gpsimd ring: small f32 tables, then the rope tables
                nc.gpsimd.dma_start(tf32_sb[:], tf32[:])
                nc.gpsimd.dma_start(tbf_sb[:], tbf[:])

                def drain_qk(ps, bias, dst, h, b, s0, tcn):
                    qsb = rtp.tile([128, 512], bf16, tag="qsb",
                                   name=f"qsb{tcn}{h}{id(dst)%97}")
                    nc.scalar.activation(qsb[:], ps[:], Ident,
                                         bias=bias[:, h:h + 1])
                    # half-swapped copy (rotate_half) via SBUF->SBUF DMA:
                    # DVE ops can't cross partition boundaries.
                    qsw = rtp.tile([128, 512], bf16, tag="qsw",
                                   name=f"qsw{tcn}{h}{id(dst)%97}")
                    nc.gpsimd.dma_start(qsw[0:64, :], qsb[64:128, :])
                    nc.gpsimd.dma_start(qsw[64:128, :], qsb[0:64, :])
                    t1 = rtp.tile([128, 512], bf16, tag="t1", name=f"t1_{tcn}{h}")
                    t2 = rtp.tile([128, 512], bf16, tag="t2", name=f"t2_{tcn}{h}")
                    nc.vector.tensor_tensor(
                        t1[:], qsb[:], cos_sb[:, s0:s0 + 512], op=mult)
                    nc.vector.tensor_tensor(
                        t2[:], qsw[:], sin_sb[:, s0:s0 + 512], op=mult)
                    nc.vector.tensor_tensor(
                        dst[b, h][:, s0:s0 + 512], t1[:], t2[:], op=add)

                pending_v = None
                for tcn in range(TC_N):
                    b = tcn // 4
                    s0 = (tcn % 4) * 512
                    if tcn == 0:
                        xt = xt0
                    else:
                        xt = xtp.tile([128, KT_N * 512], bf16, tag="xt", name=f"xt{tcn}")
                        nc.sync.dma_start(xt[:], xT[tcn * 128:(tcn + 1) * 128, :])
                        if tcn == TC_N - 1:
                            # phase-3 weights ride behind the last x chunk
                            for h in range(HPC):
                                nc.sync.dma_start(
                                    wo_sb[:, h * D:(h + 1) * D],
                                    wo[h * 128:(h + 1) * 128, :])

                    groups = ((wq_sl, bq_sb, qT), (wk_sl, bk_sb, kT))
                    if tcn == 0:
                        qk_ps = {}
                        for gi in range(2):
                            for h in range(HPC):
                                qk_ps[gi, h] = ps_qk.tile(
                                    [128, 512], f32, tag="psqk",
                                    name=f"psqk0_{gi}{h}")
                        for kq in range(4):
                            for gi, (wsl, bias, dst) in enumerate(groups):
                                for h in range(HPC):
                                    ps = qk_ps[gi, h]
                                    for k in range(kq * 4, kq * 4 + 4):
                                        nc.tensor.matmul(
                                            ps[:], wqkv_sb[:, wsl(k, h)],
                                            xt[:, k * 512:(k + 1) * 512],
                                            start=(k == 0), stop=(k == KT_N - 1))
                        for gi, (wsl, bias, dst) in enumerate(groups):
                            for h in range(HPC):
                                drain_qk(qk_ps[gi, h], bias, dst, h, b, s0, tcn)
                    else:
                        for gi, (wsl, bias, dst) in enumerate(groups):
                            for h in range(HPC):
                                ps = ps_qk.tile([128, 512], f32, tag="psqk",
                                                name=f"psqk{tcn}{gi}{h}")
                                for k in range(KT_N):
                                    nc.tensor.matmul(
                                        ps[:], wqkv_sb[:, wsl(k, h)],
                                        xt[:, k * 512:(k + 1) * 512],
                                        start=(k == 0), stop=(k == KT_N - 1))
                                drain_qk(ps, bias, dst, h, b, s0, tcn)

                    pv = [ps_v.tile([128, 512], f32, tag="psv", name=f"psv{tcn}{hf}")
                          for hf in range(2)]

                    def drain_v(hf, pv=pv, tcn=tcn, b=b):
                        for sub in range(2):
                            t_sub = hf * 2 + sub
                            jblk = (tcn % 4) * 4 + t_sub
                            for h in range(HPC):
                                nc.vector.tensor_tensor(
                                    vN[b, h][:, jblk * 128:(jblk + 1) * 128],
                                    pv[hf][:, sub * 256 + h * 128: sub * 256 + (h + 1) * 128],
                                    bvb_sb[:, h * 128:(h + 1) * 128], op=add)

                    # v chains are emitted one tcn late so the in-order PE
                    # queue never stalls ready q/k work on late wv bytes
                    def emit_v(xt=xt, pv=pv, dv=drain_v):
                        for hf in range(2):
                            for sub in range(2):
                                t_sub = hf * 2 + sub
                                for k in range(KT_N):
                                    nc.tensor.matmul(
                                        pv[hf][:, sub * 256:(sub + 1) * 256],
                                        xt[:, k * 512 + t_sub * 128: k * 512 + (t_sub + 1) * 128],
                                        wqkv_sb[:, wv_sl(k)],
                                        start=(k == 0 and sub == 0),
                                        stop=(k == KT_N - 1 and sub == 1),
                                        skip_group_check=True)
                            dv(hf)

                    if pending_v is not None:
                        pending_v()
                    pending_v = emit_v
                pending_v()

            # ================= phase 2 + 3, fine-grained interleave =======
            # psum: ps_s 2x[128,1024] (score pairs) + ps_o 2x[128,512]
            # (o-accum, ring shared with psl) + ps3 2x[128,512] (proj) = 8.
            with tc.tile_pool(name="ps_s", bufs=2, space=PSUM) as ps_s, \
                 tc.tile_pool(name="ps_o", bufs=2, space=PSUM) as ps_o, \
                 tc.tile_pool(name="ps3", bufs=2, space=PSUM) as ps3, \
                 tc.tile_pool(name="ptp", bufs=5) as ptp, \
                 tc.tile_pool(name="prp", bufs=6) as prp, \
                 tc.tile_pool(name="rrp", bufs=2) as rrp, \
                 tc.tile_pool(name="outp", bufs=4) as outp:

                gz = nc.gpsimd.to_reg(0.0)

                # ---- out-proj work queue: one item = one [128,512] psum
                # fill (2 matmuls + 1 drain) or one output DMA, pumped
                # between attention pairs so proj never backs up.
                proj_queue = []
                dcount = [0]

                def make_tile(b, tt, ncx, osb, split_dma, alt_ring=False):
                    def go():
                        if alt_ring:
                            # attention is done: borrow the dead ps_s ring so
                            # four proj tiles pipeline instead of two
                            ps = ps_s.tile([128, 1024], f32, tag="pss",
                                           name=f"ps3b_{b}{tt}{ncx}")[:, 0:512]
                        else:
                            ps = ps3.tile([128, 512], f32, tag="ps3",
                                          name=f"ps3_{b}{tt}{ncx}")
                        for hh in range(HPC):
                            nc.tensor.matmul(
                                ps[:],
                                oT[b, hh][:, tt * 128:(tt + 1) * 128],
                                wo_sb[:, hh * D + ncx * 512: hh * D + (ncx + 1) * 512],
                                start=(hh == 0), stop=(hh == 1),
                                skip_group_check=True)
                        dst = osb[:, ncx * 512:(ncx + 1) * 512]
                        # drains split ACT 2/7 : DVE 5/7, spaced
                        if dcount[0] % 7 in (0, 3):
                            nc.scalar.activation(dst, ps[:], Copy)
                        else:
                            nc.vector.tensor_copy(dst, ps[:])
                        dcount[0] += 1
                        if split_dma:
                            row0 = b * S + tt * 128
                            nc.sync.dma_start(
                                out[row0:row0 + 128, ncx * 512:(ncx + 1) * 512],
                                dst)
                    return go

                def make_dma(b, tt, osb):
                    def go():
                        row0 = b * S + tt * 128
                        nc.sync.dma_start(out[row0:row0 + 128, :], osb[:])
                    return go

                def enqueue_group(b, ic, split_dma=False):
                    for tt in range(ic * 4, ic * 4 + 4):
                        osb = outp.tile([128, D], bf16, tag="osb",
                                        name=f"osb{b}{tt}")
                        for ncx in range(NC_N):
                            proj_queue.append(
                                make_tile(b, tt, ncx, osb, split_dma))
                        if not split_dma:
                            proj_queue.append(make_dma(b, tt, osb))

                def pump(n):
                    for _ in range(n):
                        if not proj_queue:
                            return
                        proj_queue.pop(0)()

                ocount = [0]

                def attn_unit(b, h, ic):
                    njb = 4 * (ic + 1)
                    npair = njb // 2
                    pso = ps_o.tile([128, 512], f32, tag="pso",
                                    name=f"pso{b}{h}{ic}")
                    psl_holder = []
                    nlmm = [0]
                    pend_quads = []
                    prs = []

                    def emit_l(t, last):
                        if nlmm[0] == 0:
                            psl_holder.append(
                                ps_o.tile([128, 512], f32, tag="pso",
                                          name=f"psl{b}{h}{ic}"))
                        nc.tensor.matmul(psl_holder[0][:], ones_sb[:], t[:],
                                         start=(nlmm[0] == 0), stop=last,
                                         skip_group_check=True)
                        nlmm[0] += 1

                    def emit_scores(jp):
                        pss = ps_s.tile([128, 1024], f32, tag="pss",
                                        name=f"pss{b}{h}{ic}{jp}")
                        pt = ptp.tile([128, 1024], bf16, tag="pt",
                                      name=f"pt{b}{h}{ic}{jp}")
                        for i in range(2):
                            jb = 2 * jp + i
                            di = jb - 4 * ic
                            lo = max(di, 0) * 128
                            nc.tensor.matmul(
                                pss[:, i * 512 + lo:(i + 1) * 512],
                                kT[b, h][:, jb * 128:(jb + 1) * 128],
                                qT[b, h][:, ic * 512 + lo:(ic + 1) * 512],
                                start=True, stop=True)
                        pump(1)
                        if 2 * jp + 1 < 4 * ic:
                            # full pair: one wide exp over both banks
                            nc.scalar.activation(pt[:, :], pss[:, :], Exp,
                                                 scale=SCALE)
                        else:
                            for i in range(2):
                                jb = 2 * jp + i
                                di = jb - 4 * ic
                                lo = max(di, 0) * 128
                                nc.scalar.activation(
                                    pt[:, i * 512 + lo:(i + 1) * 512],
                                    pss[:, i * 512 + lo:(i + 1) * 512],
                                    Exp, scale=SCALE)
                                if di >= 0:
                                    if lo > 0:
                                        nc.gpsimd.memset(
                                            pt[:, i * 512:i * 512 + lo], 0.0)
                                    # triangle: keep where query >= key
                                    nc.gpsimd.affine_select(
                                        out=pt[:, i * 512 + lo:i * 512 + lo + 128],
                                        in_=pt[:, i * 512 + lo:i * 512 + lo + 128],
                                        pattern=[[1, 128]],
                                        compare_op=is_ge,
                                        fill=gz,
                                        base=0,
                                        channel_multiplier=-1)
                        return pt

                    def emit_o(jp, pt):
                        for i in range(2):
                            jb = 2 * jp + i
                            di = jb - 4 * ic
                            lo = max(di, 0) * 128
                            nc.tensor.matmul(
                                pso[:, lo:512],
                                vN[b, h][:, jb * 128:(jb + 1) * 128],
                                pt[:, i * 512 + lo:(i + 1) * 512],
                                start=(jb == 0), stop=(jb == njb - 1),
                                skip_group_check=True)
                        pump(2 if len(proj_queue) > 24 else 1)
                        pr = prp.tile([128, 512], bf16, tag="pr",
                                      name=f"pr{b}{h}{ic}{jp}")
                        nc.vector.tensor_tensor(pr[:], pt[:, 0:512],
                                                pt[:, 512:1024], op=add)
                        prs.append(pr)
                        if len(prs) == 2:
                            q = prp.tile([128, 512], bf16, tag="pr",
                                         name=f"qd{b}{h}{ic}{jp}")
                            nc.gpsimd.tensor_tensor(q[:], prs[0][:], prs[1][:],
                                                    op=add)
                            prs.clear()
                            pend_quads.append(q)
                        if len(pend_quads) >= 2:
                            emit_l(pend_quads.pop(0), last=False)

                    # one-pair lookahead: pair jp's scores+exp are emitted
                    # before pair jp-1's o-matmuls, so the next unit's first
                    # o-matmul never stalls on the previous oT drain and the
                    # exp pipeline stays 2 deep.
                    pending_pair = None
                    for jp in range(npair):
                        pt = emit_scores(jp)
                        if pending_pair is not None:
                            emit_o(*pending_pair)
                        pending_pair = (jp, pt)
                    emit_o(*pending_pair)
                    # decouple: a plain psum->sbuf copy frees pso right after
                    # the last o-matmul; the l/recip/normalize chain then runs
                    # off the next unit's critical path.
                    oU = rrp.tile([128, 512], f32, tag="oU", name=f"oU{b}{h}{ic}")
                    if ocount[0] % 2 == 0:
                        nc.scalar.activation(oU[:], pso[:], Copy)
                    else:
                        nc.vector.tensor_copy(oU[:], pso[:])
                    ocount[0] += 1
                    while pend_quads:
                        emit_l(pend_quads.pop(0), last=(len(pend_quads) == 0))
                    psl = psl_holder[0]
                    rr = rrp.tile([128, 512], f32, tag="rr", name=f"rr{b}{h}{ic}")
                    nc.vector.reciprocal_approx_fast(rr[:], psl[:])
                    nc.vector.tensor_tensor(
                        oT[b, h][:, ic * 512:(ic + 1) * 512], oU[:], rr[:],
                        op=mult)
                    pump(2)

                ics = list(range(IC_N - 1, -1, -1))   # largest-first
                for b in range(B):
                    for ic in ics:
                        for h in range(HPC):
                            attn_unit(b, h, ic)
                        # enqueue immediately: pumped items trail by >=1 pair
                        # anyway, which covers the oT-drain latency
                        if not (b == B - 1 and ic == 0):
                            enqueue_group(b, ic)

                # leftovers, then the final group with per-tile output DMAs
                while proj_queue:
                    proj_queue.pop(0)()
                bl = B - 1
                k = 0
                for tt in range(4):
                    osb = outp.tile([128, D], bf16, tag="osb", name=f"osbT{tt}")
                    for ncx in range(NC_N):
                        make_tile(bl, tt, ncx, osb, True, alt_ring=(k % 2 == 1))()
                        k += 1

    nc.compile()
    return nc


def _host_prep(x, w_qkv, b_qkv, w_out, b_out):
    """Build the 8 per-core input maps."""
    bf = ml_dtypes.bfloat16
    # partition-major xT: row tcn*128+p = [k, 512] line for partition p
    xTf = x.reshape(T, D).T                                  # [D, T]
    xT = np.ascontiguousarray(
        xTf.reshape(KT_N, 128, TC_N, 512).transpose(2, 1, 0, 3)
    ).reshape(TC_N * 128, KT_N * 512).astype(bf)

    def wmajor(w):
        # [D, 256] -> partition-major [128, KT_N, 256]
        return np.ascontiguousarray(
            w.reshape(KT_N, 128, HPC * DH).transpose(1, 0, 2))

    # RoPE tables: cos/sin [S, DH//2] -> stacked transposed [DH, S]
    inv_freq = 1.0 / (ROPE_BASE ** (np.arange(0, DH, 2, dtype=np.float32) / DH))
    t = np.arange(S, dtype=np.float32)
    freqs = np.outer(t, inv_freq)                       # [S, 64]
    cosT = np.cos(freqs).T.astype(np.float32)           # [64, S]
    sinT = np.sin(freqs).T.astype(np.float32)
    cos2 = np.concatenate([cosT, cosT], axis=0).astype(bf)      # [128, S]
    sin2 = np.concatenate([-sinT, sinT], axis=0).astype(bf)     # [128, S]
    tbf = np.concatenate([cos2, sin2], axis=1)                  # [128, 4096]

    in_maps = []
    for c in range(NCORES):
        h0 = c * HPC
        cols = slice(h0 * DH, (h0 + HPC) * DH)
        wq_c = wmajor(w_qkv[:, cols].astype(bf))                    # [128,16,256]
        wk_c = wmajor(w_qkv[:, D + h0 * DH: D + (h0 + HPC) * DH].astype(bf))
        wv_c = wmajor(w_qkv[:, 2 * D + h0 * DH: 2 * D + (h0 + HPC) * DH].astype(bf))
        # quarter-major pack: [128, kq, (wq 4*256 | wk 4*256 | wv 4*256)]
        wqkv = np.empty((128, 4, 3 * 1024), dtype=bf)
        for kq in range(4):
            wqkv[:, kq, 0:1024] = wq_c[:, kq * 4:(kq + 1) * 4].reshape(128, 1024)
            wqkv[:, kq, 1024:2048] = wk_c[:, kq * 4:(kq + 1) * 4].reshape(128, 1024)
            wqkv[:, kq, 2048:3072] = wv_c[:, kq * 4:(kq + 1) * 4].reshape(128, 1024)
        wqkv = np.ascontiguousarray(wqkv.reshape(128, 4 * 3072))
        wo_c = w_out[cols, :].astype(bf)
        bq_c = b_qkv[cols].reshape(HPC, DH).T.astype(np.float32)          # [128, 2]
        bk_c = b_qkv[D + h0 * DH: D + (h0 + HPC) * DH].reshape(HPC, DH).T.astype(np.float32)
        bv_c = b_qkv[2 * D + h0 * DH: 2 * D + (h0 + HPC) * DH].astype(np.float32)
        bvb_c = np.broadcast_to(bv_c[None, :], (128, HPC * DH))
        tf32 = np.ascontiguousarray(
            np.concatenate([bq_c, bk_c, bvb_c], axis=1, dtype=np.float32))
        in_maps.append({
            "xT": xT, "wqkv": wqkv, "wo": np.ascontiguousarray(wo_c),
            "tbf": tbf, "tf32": tf32,
        })
    return in_maps


def _get_program():
    if "nc" not in _CACHE:
        _CACHE["nc"] = _build_program()
    return _CACHE["nc"]


def run_on_hw(in_maps, trace=False, **kw):
    from concourse.bass_utils import run_bass_kernel_spmd
    nc = _get_program()
    return run_bass_kernel_spmd(nc, in_maps, core_ids=list(range(NCORES)),
                                trace=trace, **kw)


def kernel(x, w_qkv, b_qkv, w_out, b_out):
    x = np.asarray(x, dtype=np.float32)
    w_qkv = np.asarray(w_qkv, dtype=np.float32)
    b_qkv = np.asarray(b_qkv, dtype=np.float32)
    w_out = np.asarray(w_out, dtype=np.float32)
    b_out = np.asarray(b_out, dtype=np.float32)

    in_maps = _host_prep(x, w_qkv, b_qkv, w_out, b_out)
    res = run_on_hw(in_maps)
    acc = np.zeros((T, D), dtype=np.float32)
    for c in range(NCORES):
        acc += res.results[c]["out"].astype(np.float32)
    acc += b_out[None, :]
    return acc.reshape(B, S, D)


# revision 17
# speedup vs baseline: 1.0395x; 1.0395x over previous
"""Multi-head attention (B=2, S=2048, D=2048, H=16, RoPE, causal) on 8 TRN2 cores.

Sharding: tensor-parallel over heads (2 heads/core) x batch as data.  Each core:
  phase 1: qkv projection for its 2 heads (both batches), RoPE fused into drain.
           qT,kT produced transposed [Dh, S]; v produced natural [S, Dh].
  phase 2: causal attention per (b,h) pair: s^T = kT.T @ qT blocks -> exp ->
           mask -> oT += v.T @ pT, row-sums l += ones.T @ quad-sums (PSUM).
  phase 3: partial out-proj: out_partial = sum_h diag(1/l_h) oT_h.T @ Wout_h.
Host sums the 8 partial outputs and adds b_out.

v6 design notes (evolved from the 353.6us v3 baseline):
  - v3's phase 2+3 was SCALAR-bound (exp + drain copies ~148us of a 155us
    window).  v6: causal-restricted diagonal blocks (-20% attn PE, -15%
    exp), exp per PAIR of key blocks over [128,1024] 2-bank psum tiles,
    triangle mask via gpsimd affine_select, strip zeroing via gpsimd
    memset, quad-sum tree (pair adds on DVE, quad adds on gpsimd) with one
    ones-matmul per quad, and out-proj drains 512-wide split ACT 3/10 /
    DVE 7/10.
  - out-proj is a work QUEUE of [128,512] psum fills (ps3 bufs=2 so fills
    pipeline behind drains) pumped 1-2 items between attention pairs --
    an earlier serial version let proj back up behind attention and ran
    the tail 50us at HAM half clock.
  - the early DMA feed is round-robin shared across in-flight transfers
    and weight tensors moved at 2KB/partition descriptor lines (~40-80
    GB/s).  v6 packs wq|wk|wv quarter-major into ONE dram tensor (6KB
    lines, one transfer per quarter) and the rope/bias tables into two
    packed tensors; tcn0 runs quarter-major (4 k-steps per chain per
    quarter) so real matmuls start at ~12us instead of ~21us.
"""

import numpy as np
import ml_dtypes

B, S, D = 2, 2048, 2048
H, DH = 16, 128
NCORES = 8
HPC = H // NCORES          # heads per core
T = B * S                  # 4096 tokens
SCALE = 1.0 / float(np.sqrt(DH))
ROPE_BASE = 10000.0

TC_N = T // 512            # 8 token chunks of 512 (phase 1)
KT_N = D // 128            # 16 contraction tiles
JB_N = S // 128            # 16 key blocks per sequence
IC_N = S // 512            # 4 query chunks per sequence
TT_N = S // 128            # 16 token tiles per batch (phase 3)
NC_N = D // 512            # 4 out-column chunks

_CACHE = {}


def _build_program():
    import concourse.bacc as bacc
    import concourse.mybir as mybir
    import concourse.tile as tile
    import concourse.bass as bass

    f32 = mybir.dt.float32
    bf16 = mybir.dt.bfloat16
    add = mybir.AluOpType.add
    mult = mybir.AluOpType.mult
    is_ge = mybir.AluOpType.is_ge
    Exp = mybir.ActivationFunctionType.Exp
    Copy = mybir.ActivationFunctionType.Copy
    Ident = mybir.ActivationFunctionType.Identity
    PSUM = bass.MemorySpace.PSUM

    nc = bacc.Bacc("TRN2", target_bir_lowering=False, debug=False)

    # partition-major x: row tcn*128+p holds token chunk tcn's per-partition
    # line [k, 512] (16KB contiguous per partition -> fat DMA descriptors)
    xT = nc.dram_tensor("xT", [TC_N * 128, KT_N * 512], bf16, kind="ExternalInput")
    # all projection weights, quarter-major: quarter kq occupies cols
    # [kq*3072,(kq+1)*3072) = [wq 1024 | wk 1024 | wv 1024], so one DMA per
    # quarter moves 6KB/partition contiguous lines
    wqkv = nc.dram_tensor("wqkv", [128, 4 * 3072], bf16, kind="ExternalInput")
    wo = nc.dram_tensor("wo", [HPC * DH, D], bf16, kind="ExternalInput")
    # packed tables: bf16 [cos 2048 | sin 2048]; f32 [bq 2 | bk 2 | bvb 256]
    tbf = nc.dram_tensor("tbf", [128, 4096], bf16, kind="ExternalInput")
    tf32 = nc.dram_tensor("tf32", [128, 260], f32, kind="ExternalInput")
    out = nc.dram_tensor("out", [T, D], bf16, kind="ExternalOutput")

    def wq_sl(k, h):
        kq, j = divmod(k, 4)
        c = kq * 3072 + j * 256 + h * 128
        return slice(c, c + 128)

    def wk_sl(k, h):
        kq, j = divmod(k, 4)
        c = kq * 3072 + 1024 + j * 256 + h * 128
        return slice(c, c + 128)

    def wv_sl(k):
        kq, j = divmod(k, 4)
        c = kq * 3072 + 2048 + j * 256
        return slice(c, c + 256)

    with tile.TileContext(nc) as tc:
        with tc.tile_pool(name="persist", bufs=1) as pp:
            # --- resident weights/constants ---
            wqkv_sb = pp.tile([128, 4 * 3072], bf16, tag="wqkv_sb", name="wqkv_sb")
            wo_sb = pp.tile([128, HPC * D], bf16, tag="wo_sb", name="wo_sb")
            tbf_sb = pp.tile([128, 4096], bf16, tag="tbf_sb", name="tbf_sb")
            tf32_sb = pp.tile([128, 260], f32, tag="tf32_sb", name="tf32_sb")
            cos_sb = tbf_sb[:, 0:2048]
            sin_sb = tbf_sb[:, 2048:4096]
            bq_sb = tf32_sb[:, 0:HPC]
            bk_sb = tf32_sb[:, HPC:2 * HPC]
            bvb_sb = tf32_sb[:, 2 * HPC:2 * HPC + HPC * DH]
            # all-ones stationary: ones128.T @ t replicates colsums to all
            # 128 PSUM partitions -> denominator tile needs no broadcast
            ones_sb = pp.tile([128, 128], bf16, tag="ones_sb", name="ones_sb")
            nc.vector.memset(ones_sb[:], 1.0)

            # --- per-(b,h) persistent tensors ---
            qT, kT, vN, oT = {}, {}, {}, {}
            for b in range(B):
                for h in range(HPC):
                    qT[b, h] = pp.tile([128, S], bf16, tag=f"qT{b}{h}", name=f"qT{b}{h}")
                    kT[b, h] = pp.tile([128, S], bf16, tag=f"kT{b}{h}", name=f"kT{b}{h}")
                    vN[b, h] = pp.tile([128, S], bf16, tag=f"vN{b}{h}", name=f"vN{b}{h}")
                    oT[b, h] = pp.tile([128, S], bf16, tag=f"oT{b}{h}", name=f"oT{b}{h}")

            # ================= phase 1: qkv projection =================
            with tc.tile_pool(name="xtp", bufs=4) as xtp, \
                 tc.tile_pool(name="ps_qk", bufs=4, space=PSUM) as ps_qk, \
                 tc.tile_pool(name="ps_v", bufs=4, space=PSUM) as ps_v, \
                 tc.tile_pool(name="rtp", bufs=4) as rtp:
                # tcn0 arrives as 4 quarter-chunks; chains run quarter-major
                # so the PE starts as soon as quarter 0 lands.
                xt0 = xtp.tile([128, KT_N * 512], bf16, tag="xt", name="xt0")
                for kq in range(4):
                    nc.sync.dma_start(
                        xt0[:, kq * 2048:(kq + 1) * 2048],
                        xT[0:128, kq * 2048:(kq + 1) * 2048])
                # HAM warm-up bridging the NEFF preamble (~8us) to the first
                # quarter of data (~12us); real chains then take over.
                pwm = ps_v.tile([128, 512], f32, tag="psv", name="pwm")
                for _ in range(64):
                    nc.tensor.matmul(pwm[:, 0:128], ones_sb[:], ones_sb[:],
                                     start=True, stop=True)
                # scalar ring: one fat transfer per weight quarter
                for kq in range(4):
                    nc.scalar.dma_start(
                        wqkv_sb[:, kq * 3072:(kq + 1) * 3072],
                        wqkv[:, kq * 3072:(kq + 1) * 3072])
                # gpsimd ring: small f32 tables, then the rope tables
                nc.gpsimd.dma_start(tf32_sb[:], tf32[:])
                nc.gpsimd.dma_start(tbf_sb[:], tbf[:])

                def drain_qk(ps, bias, dst, h, b, s0, tcn):
                    qsb = rtp.tile([128, 512], bf16, tag="qsb",
                                   name=f"qsb{tcn}{h}{id(dst)%97}")
                    nc.scalar.activation(qsb[:], ps[:], Ident,
                                         bias=bias[:, h:h + 1])
                    # half-swapped copy (rotate_half) via SBUF->SBUF DMA:
                    # DVE ops can't cross partition boundaries.
                    qsw = rtp.tile([128, 512], bf16, tag="qsw",
                                   name=f"qsw{tcn}{h}{id(dst)%97}")
                    nc.gpsimd.dma_start(qsw[0:64, :], qsb[64:128, :])
                    nc.gpsimd.dma_start(qsw[64:128, :], qsb[0:64, :])
                    t1 = rtp.tile([128, 512], bf16, tag="t1", name=f"t1_{tcn}{h}")
                    t2 = rtp.tile([128, 512], bf16, tag="t2", name=f"t2_{tcn}{h}")
                    nc.vector.tensor_tensor(
                        t1[:], qsb[:], cos_sb[:, s0:s0 + 512], op=mult)
                    nc.vector.tensor_tensor(
                        t2[:], qsw[:], sin_sb[:, s0:s0 + 512], op=mult)
                    nc.vector.tensor_tensor(
                        dst[b, h][:, s0:s0 + 512], t1[:], t2[:], op=add)

                pending_v = None
                for tcn in range(TC_N):
                    b = tcn // 4
                    s0 = (tcn % 4) * 512
                    if tcn == 0:
                        xt = xt0
                    else:
                        xt = xtp.tile([128, KT_N * 512], bf16, tag="xt", name=f"xt{tcn}")
                        nc.sync.dma_start(xt[:], xT[tcn * 128:(tcn + 1) * 128, :])
                        if tcn == TC_N - 1:
                            # phase-3 weights ride behind the last x chunk
                            for h in range(HPC):
                                nc.sync.dma_start(
                                    wo_sb[:, h * D:(h + 1) * D],
                                    wo[h * 128:(h + 1) * 128, :])

                    groups = ((wq_sl, bq_sb, qT), (wk_sl, bk_sb, kT))
                    if tcn == 0:
                        qk_ps = {}
                        for gi in range(2):
                            for h in range(HPC):
                                qk_ps[gi, h] = ps_qk.tile(
                                    [128, 512], f32, tag="psqk",
                                    name=f"psqk0_{gi}{h}")
                        for kq in range(4):
                            for gi, (wsl, bias, dst) in enumerate(groups):
                                for h in range(HPC):
                                    ps = qk_ps[gi, h]
                                    for k in range(kq * 4, kq * 4 + 4):
                                        nc.tensor.matmul(
                                            ps[:], wqkv_sb[:, wsl(k, h)],
                                            xt[:, k * 512:(k + 1) * 512],
                                            start=(k == 0), stop=(k == KT_N - 1))
                        for gi, (wsl, bias, dst) in enumerate(groups):
                            for h in range(HPC):
                                drain_qk(qk_ps[gi, h], bias, dst, h, b, s0, tcn)
                    else:
                        for gi, (wsl, bias, dst) in enumerate(groups):
                            for h in range(HPC):
                                ps = ps_qk.tile([128, 512], f32, tag="psqk",
                                                name=f"psqk{tcn}{gi}{h}")
                                for k in range(KT_N):
                                    nc.tensor.matmul(
                                        ps[:], wqkv_sb[:, wsl(k, h)],
                                        xt[:, k * 512:(k + 1) * 512],
                                        start=(k == 0), stop=(k == KT_N - 1))
                                drain_qk(ps, bias, dst, h, b, s0, tcn)

                    pv = [ps_v.tile([128, 512], f32, tag="psv", name=f"psv{tcn}{hf}")
                          for hf in range(2)]

                    def drain_v(hf, pv=pv, tcn=tcn, b=b):
                        for sub in range(2):
                            t_sub = hf * 2 + sub
                            jblk = (tcn % 4) * 4 + t_sub
                            for h in range(HPC):
                                nc.vector.tensor_tensor(
                                    vN[b, h][:, jblk * 128:(jblk + 1) * 128],
                                    pv[hf][:, sub * 256 + h * 128: sub * 256 + (h + 1) * 128],
                                    bvb_sb[:, h * 128:(h + 1) * 128], op=add)

                    # v chains are emitted one tcn late so the in-order PE
                    # queue never stalls ready q/k work on late wv bytes
                    def emit_v(xt=xt, pv=pv, dv=drain_v):
                        for hf in range(2):
                            for sub in range(2):
                                t_sub = hf * 2 + sub
                                for k in range(KT_N):
                                    nc.tensor.matmul(
                                        pv[hf][:, sub * 256:(sub + 1) * 256],
                                        xt[:, k * 512 + t_sub * 128: k * 512 + (t_sub + 1) * 128],
                                        wqkv_sb[:, wv_sl(k)],
                                        start=(k == 0 and sub == 0),
                                        stop=(k == KT_N - 1 and sub == 1),
                                        skip_group_check=True)
                            dv(hf)

                    if pending_v is not None:
                        pending_v()
                    pending_v = emit_v
                pending_v()

            # ================= phase 2 + 3, fine-grained interleave =======
            # psum: ps_s 2x[128,1024] (score pairs) + ps_o 2x[128,512]
            # (o-accum, ring shared with psl) + ps3 2x[128,512] (proj) = 8.
            with tc.tile_pool(name="ps_s", bufs=2, space=PSUM) as ps_s, \
                 tc.tile_pool(name="ps_o", bufs=2, space=PSUM) as ps_o, \
                 tc.tile_pool(name="ps3", bufs=2, space=PSUM) as ps3, \
                 tc.tile_pool(name="ptp", bufs=5) as ptp, \
                 tc.tile_pool(name="prp", bufs=6) as prp, \
                 tc.tile_pool(name="rrp", bufs=2) as rrp, \
                 tc.tile_pool(name="outp", bufs=4) as outp:

                gz = nc.gpsimd.to_reg(0.0)

                # ---- out-proj work queue: one item = one [128,512] psum
                # fill (2 matmuls + 1 drain) or one output DMA, pumped
                # between attention pairs so proj never backs up.
                proj_queue = []
                dcount = [0]

                def make_tile(b, tt, ncx, osb, split_dma, alt_ring=False):
                    def go():
                        if alt_ring:
                            # attention is done: borrow the dead ps_s ring so
                            # four proj tiles pipeline instead of two
                            ps = ps_s.tile([128, 1024], f32, tag="pss",
                                           name=f"ps3b_{b}{tt}{ncx}")[:, 0:512]
                        else:
                            ps = ps3.tile([128, 512], f32, tag="ps3",
                                          name=f"ps3_{b}{tt}{ncx}")
                        for hh in range(HPC):
                            nc.tensor.matmul(
                                ps[:],
                                oT[b, hh][:, tt * 128:(tt + 1) * 128],
                                wo_sb[:, hh * D + ncx * 512: hh * D + (ncx + 1) * 512],
                                start=(hh == 0), stop=(hh == 1),
                                skip_group_check=True)
                        dst = osb[:, ncx * 512:(ncx + 1) * 512]
                        # drains split ACT 2/7 : DVE 5/7, spaced
                        if dcount[0] % 7 in (0, 3):
                            nc.scalar.activation(dst, ps[:], Copy)
                        else:
                            nc.vector.tensor_copy(dst, ps[:])
                        dcount[0] += 1
                        if split_dma:
                            row0 = b * S + tt * 128
                            nc.sync.dma_start(
                                out[row0:row0 + 128, ncx * 512:(ncx + 1) * 512],
                                dst)
                    return go

                def make_dma(b, tt, osb):
                    def go():
                        row0 = b * S + tt * 128
                        nc.sync.dma_start(out[row0:row0 + 128, :], osb[:])
                    return go

                def enqueue_group(b, ic, split_dma=False):
                    for tt in range(ic * 4, ic * 4 + 4):
                        osb = outp.tile([128, D], bf16, tag="osb",
                                        name=f"osb{b}{tt}")
                        for ncx in range(NC_N):
                            proj_queue.append(
                                make_tile(b, tt, ncx, osb, split_dma))
                        if not split_dma:
                            proj_queue.append(make_dma(b, tt, osb))

                def pump(n):
                    for _ in range(n):
                        if not proj_queue:
                            return
                        proj_queue.pop(0)()

                ocount = [0]

                def attn_unit(b, h, ic):
                    njb = 4 * (ic + 1)
                    npair = njb // 2
                    pso = ps_o.tile([128, 512], f32, tag="pso",
                                    name=f"pso{b}{h}{ic}")
                    psl_holder = []
                    nlmm = [0]
                    pend_quads = []
                    prs = []

                    def emit_l(t, last):
                        if nlmm[0] == 0:
                            psl_holder.append(
                                ps_o.tile([128, 512], f32, tag="pso",
                                          name=f"psl{b}{h}{ic}"))
                        nc.tensor.matmul(psl_holder[0][:], ones_sb[:], t[:],
                                         start=(nlmm[0] == 0), stop=last,
                                         skip_group_check=True)
                        nlmm[0] += 1

                    def emit_scores(jp):
                        pss = ps_s.tile([128, 1024], f32, tag="pss",
                                        name=f"pss{b}{h}{ic}{jp}")
                        pt = ptp.tile([128, 1024], bf16, tag="pt",
                                      name=f"pt{b}{h}{ic}{jp}")
                        for i in range(2):
                            jb = 2 * jp + i
                            di = jb - 4 * ic
                            lo = max(di, 0) * 128
                            nc.tensor.matmul(
                                pss[:, i * 512 + lo:(i + 1) * 512],
                                kT[b, h][:, jb * 128:(jb + 1) * 128],
                                qT[b, h][:, ic * 512 + lo:(ic + 1) * 512],
                                start=True, stop=True)
                        pump(1)
                        # one wide exp over both banks; for diagonal pairs
                        # the not-computed strip columns hold stale-but-
                        # finite psum (exp of it is junk) and are zeroed by
                        # the memsets below before anything reads them.
                        nc.scalar.activation(pt[:, :], pss[:, :], Exp,
                                             scale=SCALE)
                        if 2 * jp + 1 >= 4 * ic:
                            pump(1)
                            for i in range(2):
                                jb = 2 * jp + i
                                di = jb - 4 * ic
                                lo = max(di, 0) * 128
                                if di >= 0:
                                    if lo > 0:
                                        nc.gpsimd.memset(
                                            pt[:, i * 512:i * 512 + lo], 0.0)
                                    # triangle: keep where query >= key
                                    nc.gpsimd.affine_select(
                                        out=pt[:, i * 512 + lo:i * 512 + lo + 128],
                                        in_=pt[:, i * 512 + lo:i * 512 + lo + 128],
                                        pattern=[[1, 128]],
                                        compare_op=is_ge,
                                        fill=gz,
                                        base=0,
                                        channel_multiplier=-1)
                        return pt

                    def emit_o(jp, pt):
                        for i in range(2):
                            jb = 2 * jp + i
                            di = jb - 4 * ic
                            lo = max(di, 0) * 128
                            nc.tensor.matmul(
                                pso[:, lo:512],
                                vN[b, h][:, jb * 128:(jb + 1) * 128],
                                pt[:, i * 512 + lo:(i + 1) * 512],
                                start=(jb == 0), stop=(jb == njb - 1),
                                skip_group_check=True)
                        pump(2 if len(proj_queue) > 24 else 1)
                        pr = prp.tile([128, 512], bf16, tag="pr",
                                      name=f"pr{b}{h}{ic}{jp}")
                        nc.vector.tensor_tensor(pr[:], pt[:, 0:512],
                                                pt[:, 512:1024], op=add)
                        prs.append(pr)
                        if len(prs) == 2:
                            q = prp.tile([128, 512], bf16, tag="pr",
                                         name=f"qd{b}{h}{ic}{jp}")
                            nc.vector.tensor_tensor(q[:], prs[0][:], prs[1][:],
                                                    op=add)
                            prs.clear()
                            pend_quads.append(q)
                        if len(pend_quads) >= 2:
                            emit_l(pend_quads.pop(0), last=False)

                    # one-pair lookahead: pair jp's scores+exp are emitted
                    # before pair jp-1's o-matmuls, so the next unit's first
                    # o-matmul never stalls on the previous oT drain and the
                    # exp pipeline stays 2 deep.
                    pending_pair = None
                    for jp in range(npair):
                        pt = emit_scores(jp)
                        if pending_pair is not None:
                            emit_o(*pending_pair)
                        pending_pair = (jp, pt)
                    emit_o(*pending_pair)
                    while pend_quads:
                        emit_l(pend_quads.pop(0), last=(len(pend_quads) == 0))
                    psl = psl_holder[0]
                    rr = rrp.tile([128, 512], f32, tag="rr", name=f"rr{b}{h}{ic}")
                    nc.vector.reciprocal_approx_fast(rr[:], psl[:])
                    nc.vector.tensor_tensor(
                        oT[b, h][:, ic * 512:(ic + 1) * 512], pso[:], rr[:],
                        op=mult)
                    pump(2)

                ics = list(range(IC_N - 1, -1, -1))   # largest-first
                for b in range(B):
                    for ic in ics:
                        for h in range(HPC):
                            attn_unit(b, h, ic)
                        # enqueue immediately: pumped items trail by >=1 pair
                        # anyway, which covers the oT-drain latency
                        if not (b == B - 1 and ic == 0):
                            enqueue_group(b, ic)

                # leftovers, then the final group with per-tile output DMAs
                while proj_queue:
                    proj_queue.pop(0)()
                bl = B - 1
                k = 0
                for tt in range(4):
                    osb = outp.tile([128, D], bf16, tag="osb", name=f"osbT{tt}")
                    for ncx in range(NC_N):
                        make_tile(bl, tt, ncx, osb, True, alt_ring=(k % 2 == 1))()
                        k += 1

    nc.compile()
    return nc


def _host_prep(x, w_qkv, b_qkv, w_out, b_out):
    """Build the 8 per-core input maps."""
    bf = ml_dtypes.bfloat16
    # partition-major xT: row tcn*128+p = [k, 512] line for partition p
    xTf = x.reshape(T, D).T                                  # [D, T]
    xT = np.ascontiguousarray(
        xTf.reshape(KT_N, 128, TC_N, 512).transpose(2, 1, 0, 3)
    ).reshape(TC_N * 128, KT_N * 512).astype(bf)

    def wmajor(w):
        # [D, 256] -> partition-major [128, KT_N, 256]
        return np.ascontiguousarray(
            w.reshape(KT_N, 128, HPC * DH).transpose(1, 0, 2))

    # RoPE tables: cos/sin [S, DH//2] -> stacked transposed [DH, S]
    inv_freq = 1.0 / (ROPE_BASE ** (np.arange(0, DH, 2, dtype=np.float32) / DH))
    t = np.arange(S, dtype=np.float32)
    freqs = np.outer(t, inv_freq)                       # [S, 64]
    cosT = np.cos(freqs).T.astype(np.float32)           # [64, S]
    sinT = np.sin(freqs).T.astype(np.float32)
    cos2 = np.concatenate([cosT, cosT], axis=0).astype(bf)      # [128, S]
    sin2 = np.concatenate([-sinT, sinT], axis=0).astype(bf)     # [128, S]
    tbf = np.concatenate([cos2, sin2], axis=1)                  # [128, 4096]

    in_maps = []
    for c in range(NCORES):
        h0 = c * HPC
        cols = slice(h0 * DH, (h0 + HPC) * DH)
        wq_c = wmajor(w_qkv[:, cols].astype(bf))                    # [128,16,256]
        wk_c = wmajor(w_qkv[:, D + h0 * DH: D + (h0 + HPC) * DH].astype(bf))
        wv_c = wmajor(w_qkv[:, 2 * D + h0 * DH: 2 * D + (h0 + HPC) * DH].astype(bf))
        # quarter-major pack: [128, kq, (wq 4*256 | wk 4*256 | wv 4*256)]
        wqkv = np.empty((128, 4, 3 * 1024), dtype=bf)
        for kq in range(4):
            wqkv[:, kq, 0:1024] = wq_c[:, kq * 4:(kq + 1) * 4].reshape(128, 1024)
            wqkv[:, kq, 1024:2048] = wk_c[:, kq * 4:(kq + 1) * 4].reshape(128, 1024)
            wqkv[:, kq, 2048:3072] = wv_c[:, kq * 4:(kq + 1) * 4].reshape(128, 1024)
        wqkv = np.ascontiguousarray(wqkv.reshape(128, 4 * 3072))
        wo_c = w_out[cols, :].astype(bf)
        bq_c = b_qkv[cols].reshape(HPC, DH).T.astype(np.float32)          # [128, 2]
        bk_c = b_qkv[D + h0 * DH: D + (h0 + HPC) * DH].reshape(HPC, DH).T.astype(np.float32)
        bv_c = b_qkv[2 * D + h0 * DH: 2 * D + (h0 + HPC) * DH].astype(np.float32)
        bvb_c = np.broadcast_to(bv_c[None, :], (128, HPC * DH))
        tf32 = np.ascontiguousarray(
            np.concatenate([bq_c, bk_c, bvb_c], axis=1, dtype=np.float32))
        in_maps.append({
            "xT": xT, "wqkv": wqkv, "wo": np.ascontiguousarray(wo_c),
            "tbf": tbf, "tf32": tf32,
        })
    return in_maps


def _get_program():
    if "nc" not in _CACHE:
        _CACHE["nc"] = _build_program()
    return _CACHE["nc"]


def run_on_hw(in_maps, trace=False, **kw):
    from concourse.bass_utils import run_bass_kernel_spmd
    nc = _get_program()
    return run_bass_kernel_spmd(nc, in_maps, core_ids=list(range(NCORES)),
                                trace=trace, **kw)


def kernel(x, w_qkv, b_qkv, w_out, b_out):
    x = np.asarray(x, dtype=np.float32)
    w_qkv = np.asarray(w_qkv, dtype=np.float32)
    b_qkv = np.asarray(b_qkv, dtype=np.float32)
    w_out = np.asarray(w_out, dtype=np.float32)
    b_out = np.asarray(b_out, dtype=np.float32)

    in_maps = _host_prep(x, w_qkv, b_qkv, w_out, b_out)
    res = run_on_hw(in_maps)
    acc = np.zeros((T, D), dtype=np.float32)
    for c in range(NCORES):
        acc += res.results[c]["out"].astype(np.float32)
    acc += b_out[None, :]
    return acc.reshape(B, S, D)
